# revision 16
# baseline (speedup 1.0000x reference)
"""AttentionFreeTransformer on 8 trn2 NeuronCores.

Sharding: batch b -> core pair (2b, 2b+1); each core owns half the sequence
(T = S/2 tokens). The AFT cumsum couples the sequence dim only through the
running per-channel totals, so the pair exchanges one [D] vector per cumsum'd
tensor via a tiny pair-wise AllReduce (masked so the first half contributes
and the second half applies).

On-chip layout is channel-major [c, t] everywhere, which makes every matmul
operand load natural (weights are pre-transposed AND pre-tiled on the host so
every DMA is one large contiguous read per partition) and the seq cumsum a
DVE prefix scan along the free dim. No on-chip transposes at all.

  matmul1: qkv^T[c,t] = sum_d w_qkvT[d,c] * x^T[d,t]  (rms(x) folded into the
           psum drain as a per-token scale, since rms commutes with matmul)
  middle:  rms(q)/rms(k) cross-partition sums via ones-lhsT matmuls;
           exp/sigmoid/silu on ACT; cumsum scans + 1/(wcum+eps) ride in the
           DVE slack of the matmul1 Q loop (first token chunk) and in the
           matmul2 slack (second chunk) so the post-matmul1 tail is tiny.
  matmul2: uv^T[f,t] = sum_d w_swigluT[d,f] * y^T[d,t]; h^T = u*silu(g)
           (fp8 DoubleRow: 2 contraction rows per cell)
  matmul3: out[t,d] = sum_f h^T[f,t] * w_outT[f,d] + x  (token-major psum,
           so the residual add and the output DMA are both natural)
"""

import os
import sys

for _p in ("/opt/trn_rl_repo", "/root/.axon_site/_ro/trn_rl_repo"):
    if os.path.isdir(_p) and _p not in sys.path:
        sys.path.append(_p)

import numpy as np
import ml_dtypes

import concourse.bass as bass
import concourse.mybir as mybir
import concourse.tile as tile
from concourse import bacc
from concourse.bass_utils import run_bass_kernel_spmd

F32 = mybir.dt.float32
BF16 = mybir.dt.bfloat16
FP8 = mybir.dt.float8e4
MSCALE = 32.0  # fp8 weight pre-scale for w_swiglu/w_out (undone at drains)
AF = mybir.ActivationFunctionType
ALU = mybir.AluOpType

EPS = 1.1920929e-07  # torch rms_norm eps=None -> finfo(float32).eps
P = 128
N_CORES = 8
FP8_MLP = True


def build_nc(B, S, D, DFF, use_silu=True, fp8_mlp=FP8_MLP):
    """Build the single-core SPMD program (same on all 8 cores)."""
    assert B * 2 == N_CORES
    T = S // 2             # tokens per core
    TD = D // P            # d-chunks (contraction)
    C3 = 3 * D
    NCT = C3 // P          # matmul1 c-tiles
    FU = DFF // P          # u f-tiles (same count for g)
    FH = FU // 2
    TC = min(512, T)       # token chunk for matmul free dim
    NT = T // TC           # token chunks
    KG = min(8, FU)        # matmul3 k-group size
    DC = min(512, D)       # matmul3 d-chunk
    ND = D // DC
    assert T % P == 0 and D % P == 0 and DFF % P == 0 and FU % KG == 0
    assert NT == 2  # the tci=1 scan-deferral below assumes two token chunks

    nc = bacc.Bacc("TRN2", target_bir_lowering=False, debug=False,
                   num_devices=N_CORES)

    MLPDT = FP8 if fp8_mlp else BF16
    # Host-pretiled inputs: per (partition, tile) the data is contiguous in
    # DRAM, so each DMA is a single large read per partition line.
    xT_d = nc.dram_tensor("xT", [P, TD, T], BF16, kind="ExternalInput")
    xres_d = nc.dram_tensor("xres", [T, D], F32, kind="ExternalInput")
    wq_d = nc.dram_tensor("wqkvT", [P, NCT, TD, P], BF16, kind="ExternalInput")
    ws_d = nc.dram_tensor("wsT", [P, 2 * FU, TD, P], MLPDT,
                          kind="ExternalInput")
    wo_d = nc.dram_tensor("woT", [P, ND, FU, DC], MLPDT, kind="ExternalInput")
    mask_d = nc.dram_tensor("mask", [1, 2], F32, kind="ExternalInput")
    out_d = nc.dram_tensor("out", [T, D], F32, kind="ExternalOutput")

    cc_in = nc.dram_tensor("cc_in", [P, 2 * TD], F32)
    cc_out = nc.dram_tensor("cc_out", [P, 2 * TD], F32)

    xT_v = xT_d.ap()                                         # [P, TD, T]
    wq_v = wq_d.ap()                                         # [P, NCT, TD, P]
    ws_v = ws_d.ap()                                         # [P, 2FU, TD, P]
    wo_v = wo_d.ap()                                         # [P, ND, FU, DC]
    xr_v = xres_d.ap().rearrange("(o p) d -> p o d", p=P)    # [P, T//P, D]
    out_v = out_d.ap().rearrange("(o p) d -> p o d", p=P)    # [P, T//P, D]

    with tile.TileContext(nc) as tc:
        persist = tc.alloc_tile_pool(name="persist", bufs=1)

        ones_col = persist.tile([P, 1], BF16, name="ones_col")
        nc.vector.memset(ones_col[:], 1.0)
        ones_rep = persist.tile([P, P], BF16, name="ones_rep")
        nc.vector.memset(ones_rep[:], 1.0)
        mask_rep = persist.tile([P, 2], F32, name="mask_rep")
        nc.sync.dma_start(mask_rep[:], mask_d.ap().to_broadcast((P, 2)))

        big = tc.alloc_tile_pool(name="big", bufs=1)
        small = tc.alloc_tile_pool(name="small", bufs=1)
        poolA = tc.alloc_tile_pool(name="phaseA", bufs=1)
        # poolY holds phase-A tiles whose last READS happen in the A->B
        # boundary chain (y0 staging, invq scratch).  It sits above poolA in
        # the stack, so phase B's first pool (weights/hT) only overlaps
        # poolA's zone -- whose tiles all die at the last matmul1 MM -- and
        # the boundary chain doesn't gate the first matmul2 weight DMAs.
        poolY = tc.alloc_tile_pool(name="poolY", bufs=1)
        psA = tc.alloc_tile_pool(name="psA", bufs=1, space="PSUM")

        def ssq_rows(src_of_dk, label):
            """Cross-partition sum of squares, replicated to all partitions
            by an all-ones [P, P] lhsT: rows[ncb] = [P, TC] psum, every
            partition holding the per-token ssq row."""
            rows = [psA.tile([P, TC], F32, name=f"psr_{label}_{ncb}",
                             tag="psr", bufs=2 * NT, space="PSUM")
                    for ncb in range(NT)]
            for dk in range(TD):
                for ncb in range(NT):
                    sq = poolA.tile([P, TC], BF16,
                                    name=f"sq_{label}_{dk}_{ncb}",
                                    tag="sq", bufs=3)
                    nc.scalar.square(
                        sq[:], src_of_dk(dk)[:, ncb * TC:(ncb + 1) * TC])
                    nc.tensor.matmul(rows[ncb][:], ones_rep[:], sq[:],
                                     start=(dk == 0), stop=(dk == TD - 1))
            return rows

        def inv_chain(rows, label, out_rep, fold_invx):
            """out_rep = rsqrt(mean(rows [* invx^2]) + eps) [* invx], all
            full-width [P, TC] ops (rows are already replicated)."""
            for ncb in range(NT):
                tsl = slice(ncb * TC, (ncb + 1) * TC)
                qa = poolA.tile([P, TC], F32, name=f"qa_{label}_{ncb}",
                                tag="qch", bufs=2)
                if fold_invx:
                    nc.vector.tensor_tensor(qa[:], rows[ncb][:],
                                            invx[:, tsl], ALU.mult)
                    nc.vector.tensor_tensor(qa[:], qa[:], invx[:, tsl],
                                            ALU.mult)
                    nc.vector.tensor_scalar(qa[:], qa[:], 1.0 / D, EPS,
                                            ALU.mult, ALU.add)
                else:
                    nc.vector.tensor_scalar(qa[:], rows[ncb][:], 1.0 / D,
                                            EPS, ALU.mult, ALU.add)
                qs = poolA.tile([P, TC], F32, name=f"qs_{label}_{ncb}",
                                tag="qch", bufs=2)
                nc.scalar.sqrt(qs[:], qa[:])
                qi = poolA.tile([P, TC], F32, name=f"qi_{label}_{ncb}",
                                tag="qch", bufs=2)
                nc.vector.reciprocal_approx_fast(qi[:], qs[:])
                if fold_invx:
                    nc.vector.tensor_tensor(out_rep[:, tsl], qi[:],
                                            invx[:, tsl], ALU.mult)
                else:
                    nc.scalar.copy(out_rep[:, tsl], qi[:])

        def big_tile(name):
            return big.tile([P, TD, T], BF16, name=name, tag="big", bufs=3)

        xT_sb = poolA.tile([P, TD, T], BF16, name="xT_sb")
        for hh in range(4):
            hsl = slice(hh * (TD // 4), (hh + 1) * (TD // 4))
            nc.sync.dma_start(xT_sb[:, hsl, :], xT_v[:, hsl, :])

        x_rows = ssq_rows(lambda dk: xT_sb[:, dk, :], "x")
        invx = persist.tile([P, T], BF16, name="rep_x", tag="rep_x", bufs=1)
        inv_chain(x_rows, "x", invx, fold_invx=False)

        qT = big_tile("qT")
        kT = big_tile("kT")
        vT = big_tile("vT")

        def mm1_tiles(cts, post_cb=None, sq_rows=None):
            for ct in cts:
                wq_t = poolA.tile([P, TD, P], BF16, name=f"wq_{ct}",
                                  tag="wq", bufs=2)
                nc.sync.dma_start(wq_t[:], wq_v[:, ct])
                grp, loc = divmod(ct, TD)
                dst = (qT, kT, vT)[grp]
                for ncb in range(NT):
                    ps = psA.tile([P, TC], F32, name=f"mm1_{ct}_{ncb}",
                                  tag="mm1", bufs=4, space="PSUM")
                    for dk in range(TD):
                        nc.tensor.matmul(
                            ps[:], wq_t[:, dk, :],
                            xT_sb[:, dk, ncb * TC:(ncb + 1) * TC],
                            start=(dk == 0), stop=(dk == TD - 1))
                    if sq_rows is not None:
                        # sum-of-squares of the *raw* projection, accumulated
                        # across c-tiles on the PE; the all-ones [P, P] lhsT
                        # replicates the row to every partition, so no
                        # partition_broadcast is needed later
                        sqt = poolA.tile([P, TC], BF16,
                                         name=f"sqp_{ct}_{ncb}",
                                         tag="sqp", bufs=1)
                        nc.scalar.square(sqt[:], ps[:])
                        nc.tensor.matmul(sq_rows[ncb][:], ones_rep[:],
                                         sqt[:],
                                         start=(loc == 0),
                                         stop=(loc == TD - 1))
                    if grp == 2:
                        nc.vector.tensor_tensor(
                            dst[:, loc, ncb * TC:(ncb + 1) * TC], ps[:],
                            invx[:, ncb * TC:(ncb + 1) * TC], ALU.mult)
                    else:
                        # q/k get rms-normalized again afterwards: leave the
                        # raw projection here (ACT drain, off the DVE) and
                        # fold invx into that later scale instead
                        nc.scalar.copy(
                            dst[:, loc, ncb * TC:(ncb + 1) * TC], ps[:])
                if post_cb is not None:
                    post_cb(ct)

        # matmul1 K tiles, then the k path (exp/wv/collective overlap the V
        # and Q matmul tiles below)
        mm1_tiles(range(TD, 2 * TD))
        k_rows = ssq_rows(lambda dk: kT[:, dk, :], "k")
        invkx = persist.tile([P, T], BF16, name="rep_kx", tag="rep_kq",
                             bufs=2)
        inv_chain(k_rows, "k", invkx, fold_invx=True)
        nc.vector.tensor_tensor(
            kT[:], kT[:], invkx[:, None, :].to_broadcast((P, TD, T)),
            ALU.mult)
        w = kT  # exp in place: w aliases kT
        nc.scalar.activation(w[:], kT[:], AF.Exp)

        # matmul1 V tiles; wv + per-channel totals per c-tile in the V slack
        wv = vT  # w*v in place: wv aliases vT
        totw = persist.tile([P, 2 * TD], F32, name="totw")

        def v_post(ct):
            cl = ct - 2 * TD
            nc.vector.tensor_tensor(wv[:, cl, :], w[:, cl, :], vT[:, cl, :],
                                    ALU.mult)
            nc.vector.tensor_reduce(totw[:, cl:cl + 1], w[:, cl, :],
                                    mybir.AxisListType.X, ALU.add)
            nc.vector.tensor_reduce(totw[:, TD + cl:TD + cl + 1],
                                    wv[:, cl, :],
                                    mybir.AxisListType.X, ALU.add)

        mm1_tiles(range(2 * TD, 3 * TD), post_cb=v_post)

        # pair-wise carry exchange (fires during the Q matmul tiles)
        cc_sb = persist.tile([P, 2 * TD], F32, name="cc_sb")
        nc.vector.tensor_scalar_mul(cc_sb[:], totw[:], mask_rep[:, 0:1])
        nc.sync.dma_start(cc_in.ap(), cc_sb[:])
        nc.gpsimd.collective_compute(
            "AllReduce", ALU.add,
            replica_groups=[[2 * b, 2 * b + 1] for b in range(B)],
            ins=[cc_in.ap().opt()], outs=[cc_out.ap().opt()])
        carry_raw = persist.tile([P, 2 * TD], F32, name="carry_raw")
        nc.sync.dma_start(carry_raw[:], cc_out.ap())
        carry = persist.tile([P, 2 * TD], F32, name="carry")
        nc.vector.tensor_scalar_mul(carry[:], carry_raw[:], mask_rep[:, 1:2])
        # fold the +1e-6 denominator guard into the w-scan's initial value
        carry_eps = persist.tile([P, TD], F32, name="carry_eps")
        nc.vector.tensor_scalar_add(carry_eps[:], carry[:, 0:TD], 1e-6)

        # per-(tchunk, c-tile) y tiles: kvcum/(wcum+eps), later multiplied by
        # sigmoid(rms(q)) in place.  Separate tile sets per token chunk so the
        # tci=1 scan work can run in phase B while matmul2 reads tci=0 tiles.
        lastw = persist.tile([P, TD], F32, name="lastw")
        lastkv = persist.tile([P, TD], F32, name="lastkv")
        y_t = [[small.tile([P, 2, TC], MLPDT, name=f"y{tci}_{cp}",
                           tag="ytile", bufs=NT * (TD // 2))
                for cp in range(TD // 2)] for tci in range(NT)]

        def y_slice(tci, ct):
            return y_t[tci][ct // 2][:, ct % 2, :]

        y0_stage = {}

        def mid_piece(tci, ct):
            tsl = slice(tci * TC, (tci + 1) * TC)
            wc = small.tile([P, TC], F32, name=f"wc_{tci}_{ct}",
                            tag="mid", bufs=4)
            init_w = (carry_eps[:, ct:ct + 1] if tci == 0
                      else lastw[:, ct:ct + 1])
            nc.vector.tensor_tensor_scan(
                wc[:], w[:, ct, tsl], w[:, ct, tsl], init_w,
                ALU.add, ALU.bypass)
            kv = small.tile([P, TC], F32, name=f"kv_{tci}_{ct}",
                            tag="mid", bufs=4)
            init_kv = (carry[:, TD + ct:TD + ct + 1] if tci == 0
                       else lastkv[:, ct:ct + 1])
            nc.vector.tensor_tensor_scan(
                kv[:], wv[:, ct, tsl], wv[:, ct, tsl], init_kv,
                ALU.add, ALU.bypass)
            if tci + 1 < NT:
                nc.vector.tensor_copy(lastw[:, ct:ct + 1], wc[:, TC - 1:TC])
                nc.vector.tensor_copy(lastkv[:, ct:ct + 1], kv[:, TC - 1:TC])
            rcp = small.tile([P, TC], F32, name=f"rcp_{tci}_{ct}",
                             tag="mid", bufs=4)
            nc.vector.reciprocal_approx_fast(rcp[:], wc[:])
            if tci == 0:
                # stage kvcum/wcum in bf16 (poolY, dies at the sigmoid
                # multiply) so y is quantized to fp8 only once
                st = poolY.tile([P, TC], BF16, name=f"y0s_{ct}", tag="y0s",
                                bufs=TD)
                nc.vector.tensor_tensor(st[:], kv[:], rcp[:], ALU.mult)
                y0_stage[ct] = st
            else:
                # sig is ready by the time tci=1 pieces run (phase B): fold
                # it into the reciprocal so y is fp8-quantized only once
                sr = small.tile([P, TC], F32, name=f"sr_{ct}",
                                tag="mid", bufs=4)
                nc.vector.tensor_tensor(sr[:], rcp[:],
                                        sig[:, ct, TC:2 * TC], ALU.mult)
                nc.vector.tensor_tensor(y_slice(1, ct), kv[:], sr[:],
                                        ALU.mult)

        q_rows = [psA.tile([P, TC], F32, name=f"psr_q_{ncb}",
                           tag="psr", bufs=2 * NT, space="PSUM")
                  for ncb in range(NT)]
        # only the first token chunk's scans ride the Q loop; the second
        # chunk's ride the matmul2 slack in phase B (their y tiles aren't
        # read until matmul2's second tchunk pass, hundreds of us later)
        pi = 0

        def q_post(ct):
            nonlocal pi
            if pi < TD:
                mid_piece(0, pi)
                pi += 1

        mm1_tiles(range(0, TD), post_cb=q_post, sq_rows=q_rows)
        while pi < TD:
            mid_piece(0, pi)
            pi += 1

        # rms(q) from the raw-projection squares (replicated rows):
        # inv_qx = invx * rsqrt(mean(ssq_raw * invx^2) + eps)
        invqx = persist.tile([P, T], BF16, name="rep_qx", tag="rep_kq",
                             bufs=2)
        inv_chain(q_rows, "q", invqx, fold_invx=True)
        # per-c-tile pipeline (DVE mult -> ACT sigmoid -> DVE y mult) so the
        # first matmul2 tiles can start as soon as their y pairs finalize
        sig = qT
        for ct in range(TD):
            nc.vector.tensor_tensor(qT[:, ct, :], qT[:, ct, :], invqx[:],
                                    ALU.mult)
            nc.scalar.activation(sig[:, ct, :], qT[:, ct, :], AF.Sigmoid)
            nc.vector.tensor_tensor(y_slice(0, ct), y0_stage[ct][:],
                                    sig[:, ct, 0:TC], ALU.mult)

        def deferred_unit(ct):
            # tci=1 scan piece (includes its sigmoid fold), issued in phase B
            mid_piece(1, ct)

        # issue slots inside matmul2's tci=0 fj loop: every other fj from 8
        defer_at = {8 + 2 * i: i for i in range(TD)}

        poolY.release()
        poolA.release()
        psA.release()

        # ---- matmul2 (uv^T, h^T = u*silu(g)) + matmul3 (+residual) ----
        # poolB1 (weights/hT/sg) fits inside poolA's freed zone, so it only
        # waits for matmul1 to finish; poolB2 (mm3-time tiles) may overlap
        # poolY's zone and wait for the boundary chain -- harmless, those
        # tiles aren't needed until matmul3.
        poolB1 = tc.alloc_tile_pool(name="phaseB1", bufs=1)
        poolB2 = tc.alloc_tile_pool(name="phaseB2", bufs=1)
        psB = tc.alloc_tile_pool(name="psB", bufs=1, space="PSUM")
        for tci in range(NT):
            tsl = slice(tci * TC, (tci + 1) * TC)
            hT_halves = [poolB1.tile([P, FH, TC], MLPDT, name=f"hT_{tci}_{i}",
                                    tag="hT", bufs=2) for i in range(2)]

            def hT_slice(k, tt):
                return hT_halves[k // FH][:, k % FH, tt * P:(tt + 1) * P]

            for fj in range(FU):
                if tci == 0 and fj in defer_at:
                    deferred_unit(defer_at[fj])
                wu_t = poolB1.tile([P, TD, P], MLPDT, name=f"wu_{tci}_{fj}",
                                  tag="ws", bufs=3)
                nc.sync.dma_start(wu_t[:], ws_v[:, fj])
                wg_t = poolB1.tile([P, TD, P], MLPDT, name=f"wg_{tci}_{fj}",
                                  tag="ws", bufs=3)
                nc.sync.dma_start(wg_t[:], ws_v[:, FU + fj])
                psu = psB.tile([P, TC], F32, name=f"psu_{tci}_{fj}",
                               tag="mm2", bufs=3, space="PSUM")
                psg = psB.tile([P, TC], F32, name=f"psg_{tci}_{fj}",
                               tag="mm2", bufs=3, space="PSUM")
                if fp8_mlp:
                    for dk in range(0, TD, 2):
                        nc.tensor.matmul(
                            psu[:], wu_t[:, dk:dk + 2, :],
                            y_t[tci][dk // 2][:],
                            start=(dk == 0), stop=(dk == TD - 2),
                            perf_mode=mybir.MatmulPerfMode.DoubleRow)
                    for dk in range(0, TD, 2):
                        nc.tensor.matmul(
                            psg[:], wg_t[:, dk:dk + 2, :],
                            y_t[tci][dk // 2][:],
                            start=(dk == 0), stop=(dk == TD - 2),
                            perf_mode=mybir.MatmulPerfMode.DoubleRow)
                else:
                    for dk in range(TD):
                        nc.tensor.matmul(psu[:], wu_t[:, dk, :],
                                         y_slice(tci, dk),
                                         start=(dk == 0), stop=(dk == TD - 1))
                    for dk in range(TD):
                        nc.tensor.matmul(psg[:], wg_t[:, dk, :],
                                         y_slice(tci, dk),
                                         start=(dk == 0), stop=(dk == TD - 1))
                sg = poolB1.tile([P, TC], BF16, name=f"sg_{tci}_{fj}",
                                tag="sg", bufs=3)
                dsc = (1.0 / MSCALE) if fp8_mlp else 1.0
                if use_silu:
                    nc.scalar.activation(sg[:], psg[:], AF.Silu, scale=dsc)
                    nc.vector.tensor_tensor(
                        hT_halves[fj // FH][:, fj % FH, :], psu[:], sg[:],
                        ALU.mult)
                else:
                    nc.scalar.activation(sg[:], psg[:], AF.Sigmoid, scale=dsc)
                    gsg = poolB1.tile([P, TC], BF16, name=f"gsg_{tci}_{fj}",
                                     tag="sg", bufs=3)
                    nc.vector.scalar_tensor_tensor(gsg[:], psg[:], dsc, sg[:],
                                                   ALU.mult, ALU.mult)
                    nc.vector.tensor_tensor(
                        hT_halves[fj // FH][:, fj % FH, :], psu[:], gsg[:],
                        ALU.mult)

            n_tt = TC // P
            for dc in range(ND):
                ps3 = [psB.tile([P, DC], F32, name=f"ps3_{tci}_{dc}_{tt}",
                                tag="mm3", bufs=5, space="PSUM")
                       for tt in range(n_tt)]
                for kg in range(FU // KG):
                    wo_t = poolB2.tile([P, KG, DC], MLPDT,
                                      name=f"wo_{tci}_{dc}_{kg}",
                                      tag="wo", bufs=3)
                    nc.sync.dma_start(
                        wo_t[:], wo_v[:, dc, kg * KG:(kg + 1) * KG, :])
                    for tt in range(n_tt):
                        if fp8_mlp:
                            for kk in range(0, KG, 2):
                                k = kg * KG + kk
                                nc.tensor.matmul(
                                    ps3[tt][:],
                                    hT_halves[k // FH][:,
                                                       k % FH:k % FH + 2,
                                                       tt * P:(tt + 1) * P],
                                    wo_t[:, kk:kk + 2, :],
                                    start=(k == 0), stop=(k == FU - 2),
                                    perf_mode=mybir.MatmulPerfMode.DoubleRow)
                        else:
                            for kk in range(KG):
                                k = kg * KG + kk
                                nc.tensor.matmul(
                                    ps3[tt][:], hT_slice(k, tt),
                                    wo_t[:, kk, :],
                                    start=(k == 0), stop=(k == FU - 1))
                dsl = slice(dc * DC, (dc + 1) * DC)
                for tt in range(n_tt):
                    tt_g = tci * (TC // P) + tt
                    xr_t = poolB2.tile([P, DC], F32,
                                      name=f"xr_{tci}_{dc}_{tt}",
                                      tag="xr", bufs=3)
                    nc.sync.dma_start(xr_t[:], xr_v[:, tt_g, dsl])
                    o_t = poolB2.tile([P, DC], F32, name=f"o_{tci}_{dc}_{tt}",
                                     tag="ot", bufs=2)
                    if fp8_mlp:
                        nc.vector.scalar_tensor_tensor(
                            o_t[:], ps3[tt][:], 1.0 / (MSCALE * MSCALE),
                            xr_t[:], ALU.mult, ALU.add)
                    else:
                        nc.vector.tensor_tensor(o_t[:], ps3[tt][:], xr_t[:],
                                                ALU.add)
                    nc.sync.dma_start(out_v[:, tt_g, dsl], o_t[:])

        psB.release()
        poolB2.release()
        poolB1.release()
        small.release()
        big.release()
        persist.release()

    nc.compile()
    return nc


_NC_CACHE = {}


def _get_nc(B, S, D, DFF, use_silu=True, fp8_mlp=FP8_MLP):
    key = (B, S, D, DFF, use_silu, fp8_mlp)
    if key not in _NC_CACHE:
        _NC_CACHE[key] = build_nc(B, S, D, DFF, use_silu=use_silu,
                                  fp8_mlp=fp8_mlp)
    return _NC_CACHE[key]


def make_in_maps(x, w_qkv, w_swiglu, w_out, fp8_mlp=FP8_MLP):
    B, S, D = x.shape
    DFF = w_out.shape[1]
    T = S // 2
    TD = D // P
    NCT = 3 * D // P
    FU = DFF // P
    DC = min(512, D)
    ND = D // DC
    bf = ml_dtypes.bfloat16
    # host pre-tiling: per (partition, tile) contiguous DRAM blocks
    wq_T = w_qkv.T.astype(bf)                                  # [D, 3D]
    wq_t = np.ascontiguousarray(
        wq_T.reshape(TD, P, NCT, P).transpose(1, 2, 0, 3))     # [P,NCT,TD,P]
    if fp8_mlp:
        f8 = ml_dtypes.float8_e4m3
        ws_T = (w_swiglu.T * MSCALE).astype(f8)                # [D, 2DFF]
        wo_T = (w_out.T * MSCALE).astype(f8)                   # [DFF, D]
    else:
        ws_T = w_swiglu.T.astype(bf)
        wo_T = w_out.T.astype(bf)
    ws_t = np.ascontiguousarray(
        ws_T.reshape(TD, P, 2 * FU, P).transpose(1, 2, 0, 3))  # [P,2FU,TD,P]
    wo_t = np.ascontiguousarray(
        wo_T.reshape(FU, P, ND, DC).transpose(1, 2, 0, 3))     # [P,ND,FU,DC]
    in_maps = []
    for c in range(N_CORES):
        b, h = divmod(c, 2)
        xc = x[b, h * T:(h + 1) * T]
        xTt = np.ascontiguousarray(
            xc.T.astype(bf).reshape(TD, P, T).transpose(1, 0, 2))
        in_maps.append({
            "xT": xTt,
            "xres": np.ascontiguousarray(xc, dtype=np.float32),
            "wqkvT": wq_t,
            "wsT": ws_t,
            "woT": wo_t,
            "mask": np.array([[1.0 - h, float(h)]], np.float32),
        })
    return in_maps


def assemble_out(results, B, S, D):
    T = S // 2
    out = np.empty((B, S, D), np.float32)
    for c in range(N_CORES):
        b, h = divmod(c, 2)
        out[b, h * T:(h + 1) * T] = results[c]["out"]
    return out


def kernel(x, w_qkv, w_swiglu, w_out):
    x = np.asarray(x, dtype=np.float32)
    w_qkv = np.asarray(w_qkv, dtype=np.float32)
    w_swiglu = np.asarray(w_swiglu, dtype=np.float32)
    w_out = np.asarray(w_out, dtype=np.float32)
    B, S, D = x.shape
    DFF = w_out.shape[1]
    nc = _get_nc(B, S, D, DFF)
    in_maps = make_in_maps(x, w_qkv, w_swiglu, w_out)
    res = run_bass_kernel_spmd(nc, in_maps, core_ids=list(range(N_CORES)))
    return assemble_out(res.results, B, S, D)


# revision 20
# speedup vs baseline: 1.2269x; 1.2269x over previous
"""AttentionFreeTransformer on 8 trn2 NeuronCores.

Sharding: batch b -> core pair (2b, 2b+1); each core owns half the sequence
(T = S/2 tokens). The AFT cumsum couples the sequence dim only through the
running per-channel totals, so the pair exchanges one [D] vector per cumsum'd
tensor via a tiny pair-wise AllReduce (masked so the first half contributes
and the second half applies).

On-chip layout is channel-major [c, t] everywhere, which makes every matmul
operand load natural (weights are pre-transposed AND pre-tiled on the host so
every DMA is one large contiguous read per partition) and the seq cumsum a
DVE prefix scan along the free dim. No on-chip transposes at all.

  matmul1: qkv^T[c,t] = sum_d w_qkvT[d,c] * x^T[d,t]  (rms(x) folded into the
           psum drain as a per-token scale, since rms commutes with matmul)
  middle:  rms(q)/rms(k) cross-partition sums via ones-lhsT matmuls;
           exp/sigmoid/silu on ACT; cumsum scans + 1/(wcum+eps) ride in the
           DVE slack of the matmul1 Q loop (first token chunk) and in the
           matmul2 slack (second chunk) so the post-matmul1 tail is tiny.
  matmul2: uv^T[f,t] = sum_d w_swigluT[d,f] * y^T[d,t]; h^T = u*silu(g)
           (fp8 DoubleRow: 2 contraction rows per cell)
  matmul3: out[t,d] = sum_f h^T[f,t] * w_outT[f,d] + x  (token-major psum,
           so the residual add and the output DMA are both natural)
"""

import os
import sys

for _p in ("/opt/trn_rl_repo", "/root/.axon_site/_ro/trn_rl_repo"):
    if os.path.isdir(_p) and _p not in sys.path:
        sys.path.append(_p)

import numpy as np
import ml_dtypes

import concourse.bass as bass
import concourse.mybir as mybir
import concourse.tile as tile
from concourse import bacc
from concourse.bass_utils import run_bass_kernel_spmd

F32 = mybir.dt.float32
BF16 = mybir.dt.bfloat16
FP8 = mybir.dt.float8e4
MSCALE = 32.0  # fp8 weight pre-scale for w_swiglu/w_out (undone at drains)
AF = mybir.ActivationFunctionType
ALU = mybir.AluOpType

EPS = 1.1920929e-07  # torch rms_norm eps=None -> finfo(float32).eps
P = 128
N_CORES = 8
FP8_MLP = True


def build_nc(B, S, D, DFF, use_silu=True, fp8_mlp=FP8_MLP):
    """Build the single-core SPMD program (same on all 8 cores)."""
    assert B * 2 == N_CORES
    T = S // 2             # tokens per core
    TD = D // P            # d-chunks (contraction)
    C3 = 3 * D
    NCT = C3 // P          # matmul1 c-tiles
    FU = DFF // P          # u f-tiles (same count for g)
    FH = FU // 2
    TC = min(512, T)       # token chunk for matmul free dim
    NT = T // TC           # token chunks
    KG = min(8, FU)        # matmul3 k-group size
    DC = min(512, D)       # matmul3 d-chunk
    ND = D // DC
    assert T % P == 0 and D % P == 0 and DFF % P == 0 and FU % KG == 0
    assert NT == 2  # the tci=1 scan-deferral below assumes two token chunks

    nc = bacc.Bacc("TRN2", target_bir_lowering=False, debug=False,
                   num_devices=N_CORES)

    MLPDT = FP8 if fp8_mlp else BF16
    # Host-pretiled inputs: per (partition, tile) the data is contiguous in
    # DRAM, so each DMA is a single large read per partition line.
    xT_d = nc.dram_tensor("xT", [P, TD, T], BF16, kind="ExternalInput")
    x8_d = nc.dram_tensor("xT8", [P, TD, T], FP8, kind="ExternalInput")
    xres_d = nc.dram_tensor("xres", [T, D], F32, kind="ExternalInput")
    wq_d = nc.dram_tensor("wq8T", [P, 2 * TD, TD, P], FP8,
                          kind="ExternalInput")
    wv_d = nc.dram_tensor("wvT", [P, TD, TD, P], BF16, kind="ExternalInput")
    ws_d = nc.dram_tensor("wsT", [P, 2 * FU, TD, P], MLPDT,
                          kind="ExternalInput")
    wo_d = nc.dram_tensor("woT", [P, ND, FU, DC], MLPDT, kind="ExternalInput")
    mask_d = nc.dram_tensor("mask", [1, 2], F32, kind="ExternalInput")
    out_d = nc.dram_tensor("out", [T, D], F32, kind="ExternalOutput")

    cc_in = nc.dram_tensor("cc_in", [P, 2 * TD], F32)
    cc_out = nc.dram_tensor("cc_out", [P, 2 * TD], F32)

    xT_v = xT_d.ap()                                         # [P, TD, T]
    x8_v = x8_d.ap()                                         # [P, TD, T]
    wq_v = wq_d.ap()                                         # [P, 2TD, TD, P]
    wv_v = wv_d.ap()                                         # [P, TD, TD, P]
    ws_v = ws_d.ap()                                         # [P, 2FU, TD, P]
    wo_v = wo_d.ap()                                         # [P, ND, FU, DC]
    xr_v = xres_d.ap().rearrange("(o p) d -> p o d", p=P)    # [P, T//P, D]
    out_v = out_d.ap().rearrange("(o p) d -> p o d", p=P)    # [P, T//P, D]

    with tile.TileContext(nc) as tc:
        persist = tc.alloc_tile_pool(name="persist", bufs=1)

        ones_col = persist.tile([P, 1], BF16, name="ones_col")
        nc.vector.memset(ones_col[:], 1.0)
        ones_rep = persist.tile([P, P], BF16, name="ones_rep")
        nc.vector.memset(ones_rep[:], 1.0)
        mask_rep = persist.tile([P, 2], F32, name="mask_rep")
        nc.sync.dma_start(mask_rep[:], mask_d.ap().to_broadcast((P, 2)))

        big = tc.alloc_tile_pool(name="big", bufs=1)
        small = tc.alloc_tile_pool(name="small", bufs=1)
        poolA = tc.alloc_tile_pool(name="phaseA", bufs=1)
        # poolY holds phase-A tiles whose last READS happen in the A->B
        # boundary chain (y0 staging, invq scratch).  It sits above poolA in
        # the stack, so phase B's first pool (weights/hT) only overlaps
        # poolA's zone -- whose tiles all die at the last matmul1 MM -- and
        # the boundary chain doesn't gate the first matmul2 weight DMAs.
        poolY = tc.alloc_tile_pool(name="poolY", bufs=1)
        psA = tc.alloc_tile_pool(name="psA", bufs=1, space="PSUM")

        def ssq_rows(src_of_dk, label, scale=1.0):
            """Cross-partition sum of (in*scale)^2, replicated to all
            partitions by an all-ones [P, P] lhsT.  fp8 squares are fine:
            the per-element quantization error averages out over the D-wide
            sum (and any scale washes out inside the rms rsqrt)."""
            rows = [psA.tile([P, TC], F32, name=f"psr_{label}_{ncb}",
                             tag="psr", bufs=2 * NT, space="PSUM")
                    for ncb in range(NT)]
            for dk in range(TD):
                for ncb in range(NT):
                    sq = poolA.tile([P, TC], FP8,
                                    name=f"sq_{label}_{dk}_{ncb}",
                                    tag="sq", bufs=3)
                    nc.scalar.activation(
                        sq[:], src_of_dk(dk)[:, ncb * TC:(ncb + 1) * TC],
                        AF.Square, scale=scale)
                    nc.tensor.matmul(rows[ncb][:], ones_rep[:], sq[:],
                                     start=(dk == 0), stop=(dk == TD - 1))
            return rows

        def inv_chain(rows, label, out_rep, fold_invx, mean_scale=None):
            """out_rep = rsqrt(rows * mean_scale [* invx^2] + eps) [* invx],
            all full-width [P, TC] ops (rows are already replicated)."""
            ms = (1.0 / D) if mean_scale is None else mean_scale
            for ncb in range(NT):
                tsl = slice(ncb * TC, (ncb + 1) * TC)
                qa = poolA.tile([P, TC], F32, name=f"qa_{label}_{ncb}",
                                tag="qch", bufs=2)
                if fold_invx:
                    nc.vector.tensor_tensor(qa[:], rows[ncb][:],
                                            invx[:, tsl], ALU.mult)
                    nc.vector.tensor_tensor(qa[:], qa[:], invx[:, tsl],
                                            ALU.mult)
                    nc.vector.tensor_scalar(qa[:], qa[:], ms, EPS,
                                            ALU.mult, ALU.add)
                else:
                    nc.vector.tensor_scalar(qa[:], rows[ncb][:], ms,
                                            EPS, ALU.mult, ALU.add)
                qs = poolA.tile([P, TC], F32, name=f"qs_{label}_{ncb}",
                                tag="qch", bufs=2)
                nc.scalar.sqrt(qs[:], qa[:])
                qi = poolA.tile([P, TC], F32, name=f"qi_{label}_{ncb}",
                                tag="qch", bufs=2)
                nc.vector.reciprocal_approx_fast(qi[:], qs[:])
                if fold_invx:
                    nc.vector.tensor_tensor(out_rep[:, tsl], qi[:],
                                            invx[:, tsl], ALU.mult)
                else:
                    nc.scalar.copy(out_rep[:, tsl], qi[:])

        def big_tile(name):
            return big.tile([P, TD, T], BF16, name=name, tag="big", bufs=3)

        xT8 = poolA.tile([P, TD, T], FP8, name="xT8")
        for hh in range(4):
            hsl = slice(hh * (TD // 4), (hh + 1) * (TD // 4))
            nc.sync.dma_start(xT8[:, hsl, :], x8_v[:, hsl, :])
        xT_sb = poolA.tile([P, TD, T], BF16, name="xT_sb")
        for hh in range(4):
            hsl = slice(hh * (TD // 4), (hh + 1) * (TD // 4))
            nc.sync.dma_start(xT_sb[:, hsl, :], xT_v[:, hsl, :])

        x_rows = ssq_rows(lambda dk: xT_sb[:, dk, :], "x")
        invx = persist.tile([P, T], BF16, name="rep_x", tag="rep_x", bufs=1)
        inv_chain(x_rows, "x", invx, fold_invx=False)

        qT = big_tile("qT")
        kT = big_tile("kT")
        vT = big_tile("vT")

        def mm1_tiles(cts, post_cb=None, sq_rows=None):
            for ct in cts:
                grp, loc = divmod(ct, TD)
                if grp == 2:
                    wq_t = poolA.tile([P, TD, P], BF16, name=f"wqv_{ct}",
                                      tag="wqv", bufs=2)
                    nc.sync.dma_start(wq_t[:], wv_v[:, loc])
                else:
                    # q/k thirds run fp8 DoubleRow: any per-element fp8
                    # error is washed by the rms-norms right after, and the
                    # x32 host pre-scale cancels inside rsqrt(mean(.^2))
                    wq_t = poolA.tile([P, TD, P], FP8, name=f"wq8_{ct}",
                                      tag="wq8", bufs=2)
                    nc.sync.dma_start(wq_t[:], wq_v[:, ct])
                dst = (qT, kT, vT)[grp]
                for ncb in range(NT):
                    tsl = slice(ncb * TC, (ncb + 1) * TC)
                    ps = psA.tile([P, TC], F32, name=f"mm1_{ct}_{ncb}",
                                  tag="mm1", bufs=4, space="PSUM")
                    if grp == 2:
                        for dk in range(TD):
                            nc.tensor.matmul(
                                ps[:], wq_t[:, dk, :], xT_sb[:, dk, tsl],
                                start=(dk == 0), stop=(dk == TD - 1))
                    else:
                        for dk in range(0, TD, 2):
                            nc.tensor.matmul(
                                ps[:], wq_t[:, dk:dk + 2, :],
                                xT8[:, dk:dk + 2, tsl],
                                start=(dk == 0), stop=(dk == TD - 2),
                                perf_mode=mybir.MatmulPerfMode.DoubleRow)
                    if sq_rows is not None:
                        # sum-of-squares of the *raw* projection, accumulated
                        # across c-tiles on the PE; the all-ones [P, P] lhsT
                        # replicates the row to every partition, so no
                        # partition_broadcast is needed later
                        sqt = poolA.tile([P, TC], FP8,
                                         name=f"sqp_{ct}_{ncb}",
                                         tag="sqp", bufs=1)
                        nc.scalar.activation(sqt[:], ps[:], AF.Square,
                                             scale=1.0 / MSCALE)
                        nc.tensor.matmul(sq_rows[ncb][:], ones_rep[:],
                                         sqt[:],
                                         start=(loc == 0),
                                         stop=(loc == TD - 1))
                    if grp == 2:
                        nc.vector.tensor_tensor(
                            dst[:, loc, tsl], ps[:], invx[:, tsl], ALU.mult)
                    else:
                        # q/k get rms-normalized again afterwards: leave the
                        # raw projection here (ACT drain, off the DVE) and
                        # fold invx into that later scale instead
                        nc.scalar.copy(dst[:, loc, tsl], ps[:])
                if post_cb is not None:
                    post_cb(ct)

        # matmul1 K tiles, then the k path (exp/wv/collective overlap the V
        # and Q matmul tiles below)
        mm1_tiles(range(TD, 2 * TD))
        k_rows = ssq_rows(lambda dk: kT[:, dk, :], "k",
                          scale=1.0 / MSCALE)
        invkx = persist.tile([P, T], BF16, name="rep_kx", tag="rep_kq",
                             bufs=2)
        inv_chain(k_rows, "k", invkx, fold_invx=True,
                  mean_scale=MSCALE * MSCALE / D)
        nc.vector.tensor_tensor(
            kT[:], kT[:], invkx[:, None, :].to_broadcast((P, TD, T)),
            ALU.mult)
        w = kT  # exp in place: w aliases kT
        nc.scalar.activation(w[:], kT[:], AF.Exp)

        # matmul1 V tiles; wv + per-channel totals per c-tile in the V slack
        wv = vT  # w*v in place: wv aliases vT
        totw = persist.tile([P, 2 * TD], F32, name="totw")

        def v_post(ct):
            cl = ct - 2 * TD
            nc.vector.tensor_tensor(wv[:, cl, :], w[:, cl, :], vT[:, cl, :],
                                    ALU.mult)
            nc.vector.tensor_reduce(totw[:, cl:cl + 1], w[:, cl, :],
                                    mybir.AxisListType.X, ALU.add)
            nc.vector.tensor_reduce(totw[:, TD + cl:TD + cl + 1],
                                    wv[:, cl, :],
                                    mybir.AxisListType.X, ALU.add)

        mm1_tiles(range(2 * TD, 3 * TD), post_cb=v_post)

        # pair-wise carry exchange (fires during the Q matmul tiles)
        cc_sb = persist.tile([P, 2 * TD], F32, name="cc_sb")
        nc.vector.tensor_scalar_mul(cc_sb[:], totw[:], mask_rep[:, 0:1])
        nc.sync.dma_start(cc_in.ap(), cc_sb[:])
        nc.gpsimd.collective_compute(
            "AllReduce", ALU.add,
            replica_groups=[[2 * b, 2 * b + 1] for b in range(B)],
            ins=[cc_in.ap().opt()], outs=[cc_out.ap().opt()])
        carry_raw = persist.tile([P, 2 * TD], F32, name="carry_raw")
        nc.sync.dma_start(carry_raw[:], cc_out.ap())
        carry = persist.tile([P, 2 * TD], F32, name="carry")
        nc.vector.tensor_scalar_mul(carry[:], carry_raw[:], mask_rep[:, 1:2])
        # fold the +1e-6 denominator guard into the w-scan's initial value
        carry_eps = persist.tile([P, TD], F32, name="carry_eps")
        nc.vector.tensor_scalar_add(carry_eps[:], carry[:, 0:TD], 1e-6)

        # per-(tchunk, c-tile) y tiles: kvcum/(wcum+eps), later multiplied by
        # sigmoid(rms(q)) in place.  Separate tile sets per token chunk so the
        # tci=1 scan work can run in phase B while matmul2 reads tci=0 tiles.
        lastw = persist.tile([P, TD], F32, name="lastw")
        lastkv = persist.tile([P, TD], F32, name="lastkv")
        y_t = [[small.tile([P, 2, TC], MLPDT, name=f"y{tci}_{cp}",
                           tag="ytile", bufs=NT * (TD // 2))
                for cp in range(TD // 2)] for tci in range(NT)]

        def y_slice(tci, ct):
            return y_t[tci][ct // 2][:, ct % 2, :]

        y0_stage = {}

        def mid_piece(tci, ct):
            tsl = slice(tci * TC, (tci + 1) * TC)
            wc = small.tile([P, TC], F32, name=f"wc_{tci}_{ct}",
                            tag="mid", bufs=3)
            init_w = (carry_eps[:, ct:ct + 1] if tci == 0
                      else lastw[:, ct:ct + 1])
            nc.vector.tensor_tensor_scan(
                wc[:], w[:, ct, tsl], w[:, ct, tsl], init_w,
                ALU.add, ALU.bypass)
            kv = small.tile([P, TC], F32, name=f"kv_{tci}_{ct}",
                            tag="mid", bufs=3)
            init_kv = (carry[:, TD + ct:TD + ct + 1] if tci == 0
                       else lastkv[:, ct:ct + 1])
            nc.vector.tensor_tensor_scan(
                kv[:], wv[:, ct, tsl], wv[:, ct, tsl], init_kv,
                ALU.add, ALU.bypass)
            if tci + 1 < NT:
                nc.vector.tensor_copy(lastw[:, ct:ct + 1], wc[:, TC - 1:TC])
                nc.vector.tensor_copy(lastkv[:, ct:ct + 1], kv[:, TC - 1:TC])
            rcp = small.tile([P, TC], F32, name=f"rcp_{tci}_{ct}",
                             tag="mid", bufs=3)
            nc.vector.reciprocal_approx_fast(rcp[:], wc[:])
            if tci == 0:
                # stage kvcum/wcum in bf16 (poolY, dies at the sigmoid
                # multiply) so y is quantized to fp8 only once
                st = poolY.tile([P, TC], BF16, name=f"y0s_{ct}", tag="y0s",
                                bufs=TD)
                nc.vector.tensor_tensor(st[:], kv[:], rcp[:], ALU.mult)
                y0_stage[ct] = st
            else:
                # sig is ready by the time tci=1 pieces run (phase B): fold
                # it into the reciprocal so y is fp8-quantized only once
                sr = small.tile([P, TC], F32, name=f"sr_{ct}",
                                tag="mid", bufs=3)
                nc.vector.tensor_tensor(sr[:], rcp[:],
                                        sig[:, ct, TC:2 * TC], ALU.mult)
                nc.vector.tensor_tensor(y_slice(1, ct), kv[:], sr[:],
                                        ALU.mult)

        q_rows = [psA.tile([P, TC], F32, name=f"psr_q_{ncb}",
                           tag="psr", bufs=2 * NT, space="PSUM")
                  for ncb in range(NT)]
        # only the first token chunk's scans ride the Q loop; the second
        # chunk's ride the matmul2 slack in phase B (their y tiles aren't
        # read until matmul2's second tchunk pass, hundreds of us later)
        pi = 0

        def q_post(ct):
            nonlocal pi
            if pi < TD:
                mid_piece(0, pi)
                pi += 1

        mm1_tiles(range(0, TD), post_cb=q_post, sq_rows=q_rows)
        while pi < TD:
            mid_piece(0, pi)
            pi += 1

        # rms(q) from the raw-projection squares (replicated rows):
        # inv_qx = invx * rsqrt(mean(ssq_raw * invx^2) + eps)
        invqx = persist.tile([P, T], BF16, name="rep_qx", tag="rep_kq",
                             bufs=2)
        inv_chain(q_rows, "q", invqx, fold_invx=True,
                  mean_scale=MSCALE * MSCALE / D)
        # per-c-tile pipeline (DVE mult -> ACT sigmoid -> DVE y mult) so the
        # first matmul2 tiles can start as soon as their y pairs finalize
        sig = qT
        for ct in range(TD):
            nc.vector.tensor_tensor(qT[:, ct, :], qT[:, ct, :], invqx[:],
                                    ALU.mult)
            nc.scalar.activation(sig[:, ct, :], qT[:, ct, :], AF.Sigmoid)
            nc.vector.tensor_tensor(y_slice(0, ct), y0_stage[ct][:],
                                    sig[:, ct, 0:TC], ALU.mult)

        def deferred_unit(ct):
            # tci=1 scan piece (includes its sigmoid fold), issued in phase B
            mid_piece(1, ct)

        # issue slots inside matmul2's tci=0 fj loop: every other fj from 8
        defer_at = {8 + 2 * i: i for i in range(TD)}

        poolY.release()
        poolA.release()
        psA.release()

        # ---- matmul2 (uv^T, h^T = u*silu(g)) + matmul3 (+residual) ----
        # poolB1 (weights/hT/sg) fits inside poolA's freed zone, so it only
        # waits for matmul1 to finish; poolB2 (mm3-time tiles) may overlap
        # poolY's zone and wait for the boundary chain -- harmless, those
        # tiles aren't needed until matmul3.
        poolB1 = tc.alloc_tile_pool(name="phaseB1", bufs=1)
        poolB2 = tc.alloc_tile_pool(name="phaseB2", bufs=1)
        psB = tc.alloc_tile_pool(name="psB", bufs=1, space="PSUM")
        for tci in range(NT):
            tsl = slice(tci * TC, (tci + 1) * TC)
            hT_halves = [poolB1.tile([P, FH, TC], MLPDT, name=f"hT_{tci}_{i}",
                                    tag="hT", bufs=2) for i in range(2)]

            def hT_slice(k, tt):
                return hT_halves[k // FH][:, k % FH, tt * P:(tt + 1) * P]

            for fj in range(FU):
                if tci == 0 and fj in defer_at:
                    deferred_unit(defer_at[fj])
                wu_t = poolB1.tile([P, TD, P], MLPDT, name=f"wu_{tci}_{fj}",
                                  tag="ws", bufs=3)
                nc.sync.dma_start(wu_t[:], ws_v[:, fj])
                wg_t = poolB1.tile([P, TD, P], MLPDT, name=f"wg_{tci}_{fj}",
                                  tag="ws", bufs=3)
                nc.sync.dma_start(wg_t[:], ws_v[:, FU + fj])
                psu = psB.tile([P, TC], F32, name=f"psu_{tci}_{fj}",
                               tag="mm2", bufs=3, space="PSUM")
                psg = psB.tile([P, TC], F32, name=f"psg_{tci}_{fj}",
                               tag="mm2", bufs=3, space="PSUM")
                if fp8_mlp:
                    for dk in range(0, TD, 2):
                        nc.tensor.matmul(
                            psu[:], wu_t[:, dk:dk + 2, :],
                            y_t[tci][dk // 2][:],
                            start=(dk == 0), stop=(dk == TD - 2),
                            perf_mode=mybir.MatmulPerfMode.DoubleRow)
                    for dk in range(0, TD, 2):
                        nc.tensor.matmul(
                            psg[:], wg_t[:, dk:dk + 2, :],
                            y_t[tci][dk // 2][:],
                            start=(dk == 0), stop=(dk == TD - 2),
                            perf_mode=mybir.MatmulPerfMode.DoubleRow)
                else:
                    for dk in range(TD):
                        nc.tensor.matmul(psu[:], wu_t[:, dk, :],
                                         y_slice(tci, dk),
                                         start=(dk == 0), stop=(dk == TD - 1))
                    for dk in range(TD):
                        nc.tensor.matmul(psg[:], wg_t[:, dk, :],
                                         y_slice(tci, dk),
                                         start=(dk == 0), stop=(dk == TD - 1))
                sg = poolB1.tile([P, TC], BF16, name=f"sg_{tci}_{fj}",
                                tag="sg", bufs=3)
                dsc = (1.0 / MSCALE) if fp8_mlp else 1.0
                if use_silu:
                    nc.scalar.activation(sg[:], psg[:], AF.Silu, scale=dsc)
                    nc.vector.tensor_tensor(
                        hT_halves[fj // FH][:, fj % FH, :], psu[:], sg[:],
                        ALU.mult)
                else:
                    nc.scalar.activation(sg[:], psg[:], AF.Sigmoid, scale=dsc)
                    gsg = poolB1.tile([P, TC], BF16, name=f"gsg_{tci}_{fj}",
                                     tag="sg", bufs=3)
                    nc.vector.scalar_tensor_tensor(gsg[:], psg[:], dsc, sg[:],
                                                   ALU.mult, ALU.mult)
                    nc.vector.tensor_tensor(
                        hT_halves[fj // FH][:, fj % FH, :], psu[:], gsg[:],
                        ALU.mult)

            n_tt = TC // P
            for dc in range(ND):
                ps3 = [psB.tile([P, DC], F32, name=f"ps3_{tci}_{dc}_{tt}",
                                tag="mm3", bufs=5, space="PSUM")
                       for tt in range(n_tt)]
                for kg in range(FU // KG):
                    wo_t = poolB2.tile([P, KG, DC], MLPDT,
                                      name=f"wo_{tci}_{dc}_{kg}",
                                      tag="wo", bufs=3)
                    nc.sync.dma_start(
                        wo_t[:], wo_v[:, dc, kg * KG:(kg + 1) * KG, :])
                    for tt in range(n_tt):
                        if fp8_mlp:
                            for kk in range(0, KG, 2):
                                k = kg * KG + kk
                                nc.tensor.matmul(
                                    ps3[tt][:],
                                    hT_halves[k // FH][:,
                                                       k % FH:k % FH + 2,
                                                       tt * P:(tt + 1) * P],
                                    wo_t[:, kk:kk + 2, :],
                                    start=(k == 0), stop=(k == FU - 2),
                                    perf_mode=mybir.MatmulPerfMode.DoubleRow)
                        else:
                            for kk in range(KG):
                                k = kg * KG + kk
                                nc.tensor.matmul(
                                    ps3[tt][:], hT_slice(k, tt),
                                    wo_t[:, kk, :],
                                    start=(k == 0), stop=(k == FU - 1))
                dsl = slice(dc * DC, (dc + 1) * DC)
                for tt in range(n_tt):
                    tt_g = tci * (TC // P) + tt
                    xr_t = poolB2.tile([P, DC], F32,
                                      name=f"xr_{tci}_{dc}_{tt}",
                                      tag="xr", bufs=3)
                    nc.sync.dma_start(xr_t[:], xr_v[:, tt_g, dsl])
                    o_t = poolB2.tile([P, DC], F32, name=f"o_{tci}_{dc}_{tt}",
                                     tag="ot", bufs=2)
                    if fp8_mlp:
                        nc.vector.scalar_tensor_tensor(
                            o_t[:], ps3[tt][:], 1.0 / (MSCALE * MSCALE),
                            xr_t[:], ALU.mult, ALU.add)
                    else:
                        nc.vector.tensor_tensor(o_t[:], ps3[tt][:], xr_t[:],
                                                ALU.add)
                    nc.sync.dma_start(out_v[:, tt_g, dsl], o_t[:])

        psB.release()
        poolB2.release()
        poolB1.release()
        small.release()
        big.release()
        persist.release()

    nc.compile()
    return nc


_NC_CACHE = {}


def _get_nc(B, S, D, DFF, use_silu=True, fp8_mlp=FP8_MLP):
    key = (B, S, D, DFF, use_silu, fp8_mlp)
    if key not in _NC_CACHE:
        _NC_CACHE[key] = build_nc(B, S, D, DFF, use_silu=use_silu,
                                  fp8_mlp=fp8_mlp)
    return _NC_CACHE[key]


def make_in_maps(x, w_qkv, w_swiglu, w_out, fp8_mlp=FP8_MLP):
    B, S, D = x.shape
    DFF = w_out.shape[1]
    T = S // 2
    TD = D // P
    NCT = 3 * D // P
    FU = DFF // P
    DC = min(512, D)
    ND = D // DC
    bf = ml_dtypes.bfloat16
    f8qk = ml_dtypes.float8_e4m3
    # host pre-tiling: per (partition, tile) contiguous DRAM blocks.
    # q/k weight rows go to fp8 (x32 pre-scale against subnormals; the scale
    # cancels inside the rms-norms), the v rows stay bf16.
    wqk_T = (w_qkv[:2 * D].T * MSCALE).astype(f8qk)            # [D, 2D]
    wq8_t = np.ascontiguousarray(
        wqk_T.reshape(TD, P, 2 * TD, P).transpose(1, 2, 0, 3))  # [P,2TD,TD,P]
    wv_T = w_qkv[2 * D:].T.astype(bf)                          # [D, D]
    wv_t = np.ascontiguousarray(
        wv_T.reshape(TD, P, TD, P).transpose(1, 2, 0, 3))      # [P,TD,TD,P]
    if fp8_mlp:
        f8 = ml_dtypes.float8_e4m3
        ws_T = (w_swiglu.T * MSCALE).astype(f8)                # [D, 2DFF]
        wo_T = (w_out.T * MSCALE).astype(f8)                   # [DFF, D]
    else:
        ws_T = w_swiglu.T.astype(bf)
        wo_T = w_out.T.astype(bf)
    ws_t = np.ascontiguousarray(
        ws_T.reshape(TD, P, 2 * FU, P).transpose(1, 2, 0, 3))  # [P,2FU,TD,P]
    wo_t = np.ascontiguousarray(
        wo_T.reshape(FU, P, ND, DC).transpose(1, 2, 0, 3))     # [P,ND,FU,DC]
    in_maps = []
    for c in range(N_CORES):
        b, h = divmod(c, 2)
        xc = x[b, h * T:(h + 1) * T]
        xTt = np.ascontiguousarray(
            xc.T.astype(bf).reshape(TD, P, T).transpose(1, 0, 2))
        in_maps.append({
            "xT": xTt,
            "xT8": np.ascontiguousarray(xTt).astype(f8qk),
            "xres": np.ascontiguousarray(xc, dtype=np.float32),
            "wq8T": wq8_t,
            "wvT": wv_t,
            "wsT": ws_t,
            "woT": wo_t,
            "mask": np.array([[1.0 - h, float(h)]], np.float32),
        })
    return in_maps


def assemble_out(results, B, S, D):
    T = S // 2
    out = np.empty((B, S, D), np.float32)
    for c in range(N_CORES):
        b, h = divmod(c, 2)
        out[b, h * T:(h + 1) * T] = results[c]["out"]
    return out


def kernel(x, w_qkv, w_swiglu, w_out):
    x = np.asarray(x, dtype=np.float32)
    w_qkv = np.asarray(w_qkv, dtype=np.float32)
    w_swiglu = np.asarray(w_swiglu, dtype=np.float32)
    w_out = np.asarray(w_out, dtype=np.float32)
    B, S, D = x.shape
    DFF = w_out.shape[1]
    nc = _get_nc(B, S, D, DFF)
    in_maps = make_in_maps(x, w_qkv, w_swiglu, w_out)
    res = run_bass_kernel_spmd(nc, in_maps, core_ids=list(range(N_CORES)))
    return assemble_out(res.results, B, S, D)
